# revision 37
# baseline (speedup 1.0000x reference)
"""Self-contained Trainium2 (Bass/Tile) kernel for the causal-attention module.

Problem shapes (hardcoded): x [2, 2048, 2048] fp32, rotary_emb [2048, 64] fp32,
gamma [2048] fp32, Wq [2048, 2048], Wkv [2048, 4096], Wout [2048, 2048] fp32.

Sharding: 8 NeuronCores = 2 batches (data parallel) x 4 head groups of 8
heads (tensor parallel).  Each core computes a full [2048, 2048] partial
output in bf16 (its head group's contribution through Wout's row block); the
host sums the 4 partials per batch in fp32.

Host prep: RMSNorm (gamma folded) is applied on the host and the normalized
activations are shipped pre-transposed as xn^T [dim, tok] bf16.

Per-core kernel (matmuls bf16, fp32 PSUM):
  - K^T d-major [2 heads x 64d, tok] per head-pair, weights preloaded.
    rotate_half runs on the PE via a host-sent permutation matrix.
    V natural [tok, h, 64] + ones column (softmax denominator free in the
    AV matmul).
  - Attention per 512-token i-block: scores S^T[j,i] for BOTH heads of a
    head-pair issued back-to-back into one [128,2,512] PSUM pair tile; the
    two C=64 matmuls land in disjoint PE row groups (tile_position row
    tiling) and execute concurrently.  Causal masking of diagonal blocks is
    done on the PE too: an accumulating matmul (lhsT=I, rhs=-3e29 strict
    upper triangle) seeds -inf into the masked positions, so no DVE mask
    multiplies.  exp runs on ScalarE over the full [128,1024] pair tile
    (one ACTIVATE per j-block instead of two).
  - AV accumulated in PSUM per head ([65,512], ones column = denominator).
  - Softmax denominators: the [1,512] PSUM denominator rows are copied to
    partitions 0/32 (32-aligned as DVE requires), an SBUF->SBUF DMA spreads
    them over contiguous partitions 0..7, so the DVE reciprocal runs 8
    lanes wide (vs 1 lane = 3.3us each before); 8 C=8 selector matmuls
    broadcast 1/d across partitions and a single [128,512] multiply
    normalizes both heads.
  - Software pipelining: Q projection+rotary for i-block i+1, V projection
    for i-blocks 4(i+1)..4(i+2), and the Wout projection of i-block i-2 are
    interleaved between the scores and AV matmuls so TensorE never waits on
    ScalarE's exp (keeps the PE HAM clock-gate at K=8/8).
"""

from contextlib import ExitStack

import numpy as np
import ml_dtypes

B, N, DIM = 2, 2048, 2048
HEADS_TOTAL, DH = 32, 64
N_CORES = 8
GROUPS = 4
HEADS = HEADS_TOTAL // GROUPS      # heads per core
HD = HEADS * DH                    # 512
IB = 512                           # query i-block width

_CACHED = {}


def _build():
    import concourse.tile as tile
    from concourse import mybir, bacc

    F32 = mybir.dt.float32
    BF16 = mybir.dt.bfloat16
    AF = mybir.ActivationFunctionType
    ALU = mybir.AluOpType

    n_ct = DIM // 128      # 16 contraction blocks
    n_tt = N // 128        # 16 token blocks
    n_ib = N // IB         # 4 i-blocks
    n_hb = HD // 128       # 4 head pairs
    jpi = IB // 128        # 4 j-blocks per i-block
    scale = DH ** -0.5

    nc = bacc.Bacc(None)
    xnT_d = nc.declare_dram_parameter("xnT", [DIM, N], BF16, isOutput=False)
    wq_d = nc.declare_dram_parameter("wq", [DIM, HD], BF16, isOutput=False)
    wk_d = nc.declare_dram_parameter("wk", [DIM, HD], BF16, isOutput=False)
    wv_d = nc.declare_dram_parameter("wv", [DIM, HD], BF16, isOutput=False)
    wout_d = nc.declare_dram_parameter("wout", [HD, DIM], BF16, isOutput=False)
    cosr_d = nc.declare_dram_parameter("cosr", [128, N], BF16, isOutput=False)
    sinr_d = nc.declare_dram_parameter("sinr", [128, N], BF16, isOutput=False)
    trineg_d = nc.declare_dram_parameter("trineg", [128, 128], BF16,
                                         isOutput=False)
    eye_d = nc.declare_dram_parameter("eye", [128, 128], BF16, isOutput=False)
    sel8_d = nc.declare_dram_parameter("sel8", [128, 512], BF16, isOutput=False)
    perm_d = nc.declare_dram_parameter("perm", [128, 128], BF16, isOutput=False)
    out_d = nc.declare_dram_parameter("out", [N, DIM], BF16, isOutput=True)

    ctx = ExitStack()
    with ctx:
        tc = ctx.enter_context(tile.TileContext(nc))
        pers = ctx.enter_context(tc.tile_pool(name="pers", bufs=1))
        wqp = ctx.enter_context(tc.tile_pool(name="wqp", bufs=2))
        qtp = ctx.enter_context(tc.tile_pool(name="qtp", bufs=2))
        epool = ctx.enter_context(tc.tile_pool(name="epool", bufs=3))
        rot = ctx.enter_context(tc.tile_pool(name="rot", bufs=2))
        ontp = ctx.enter_context(tc.tile_pool(name="ontp", bufs=3))
        osbp = ctx.enter_context(tc.tile_pool(name="osbp", bufs=2))
        ocp = ctx.enter_context(tc.tile_pool(name="ocp", bufs=2))
        denp = ctx.enter_context(tc.tile_pool(name="denp", bufs=2))
        ps = ctx.enter_context(tc.tile_pool(name="ps", bufs=2, space="PSUM"))
        ps_sc = ctx.enter_context(tc.tile_pool(name="pssc", bufs=2,
                                               space="PSUM"))
        ps_av = ctx.enter_context(tc.tile_pool(name="psav", bufs=1,
                                               space="PSUM"))

        dmae = [nc.sync, nc.gpsimd]

        xnT = [pers.tile([128, N], BF16, tag=f"xnT{c}", name=f"xnT{c}")
               for c in range(n_ct)]
        kt = [pers.tile([128, N], BF16, tag=f"kt{h}", name=f"kt{h}")
              for h in range(n_hb)]
        vst = [pers.tile([128, HEADS, DH + 1], BF16, tag=f"v{t}", name=f"v{t}")
               for t in range(n_tt)]
        wk_sb = pers.tile([128, n_ct, HD], BF16, tag="wk")
        wv_sb = pers.tile([128, n_ct, HD], BF16, tag="wv")
        wout_sb = pers.tile([128, n_hb, DIM], BF16, tag="wout")
        crep = pers.tile([128, N], BF16, tag="crep")
        srep = pers.tile([128, N], BF16, tag="srep")
        trineg = pers.tile([128, 128], BF16, tag="trineg")
        eye = pers.tile([128, 128], BF16, tag="eye")
        sel8 = pers.tile([128, 512], BF16, tag="sel8")
        perm = pers.tile([128, 128], BF16, tag="perm")
        onesb = pers.tile([128, 128], BF16, tag="onesb")

        # startup: xnT tiles front-loaded with just-in-time wk c-pairs (the
        # wave consumes c-pair p right after xnT[2p+1] lands); rotary tables
        # near the stream's end (first rotary fin runs just after), the
        # rest after.
        wkr = wk_d.rearrange("(c p) h -> p c h", p=128)
        # first xnT tiles lead; tiny tables (0.2 MB) right behind them so
        # the wave fins' perm matmul never queue-head-blocks on a table DMA
        # stuck behind 512KB transfers; rotary cos/sin tables mid-stream.
        nc.sync.dma_start(wk_sb[:, 0:2, :], wkr[:, 0:2, :])
        for c in range(n_ct):
            dmae[c % 2].dma_start(out=xnT[c][:],
                                  in_=xnT_d[c * 128:(c + 1) * 128, :])
            if c % 2 == 1 and c < n_ct - 1:
                p = (c + 1) // 2
                dmae[(c + 1) % 2].dma_start(wk_sb[:, 2 * p:2 * p + 2, :],
                                            wkr[:, 2 * p:2 * p + 2, :])
            if c == 3:
                nc.sync.dma_start(perm[:], perm_d[:])
                nc.gpsimd.dma_start(trineg[:], trineg_d[:])
                nc.gpsimd.dma_start(eye[:], eye_d[:])
                nc.sync.dma_start(sel8[:], sel8_d[:])
            if c == 5:
                nc.sync.dma_start(crep[:], cosr_d[:])
            if c == 6:
                nc.gpsimd.dma_start(srep[:], sinr_d[:])
        nc.gpsimd.dma_start(wv_sb[:], wv_d.rearrange("(c p) h -> p c h", p=128))
        nc.sync.dma_start(wout_sb[:], wout_d.rearrange("(g p) e -> p g e", p=128))
        nc.vector.memset(onesb[:], 1.0)
        for t in range(n_tt):
            nc.vector.memset(vst[t][:, :, DH:DH + 1], 1.0)

        def rotary(ps_ap, dsl, tcl, sq_inplace=False):
            """psum q/k AP [128, IB] -> dsl (bf16 slice), rotary applied.
            rotate_half's partition permutation runs on the PE (perm matmul);
            the sigma'd copy is consumed straight from PSUM.  sq_inplace
            reuses the source PSUM region for the permuted copy (raw already
            holds the data) so no pool slot is cycled."""
            raw = rot.tile([128, IB], BF16, tag="raw", name="raw")
            nc.vector.tensor_copy(raw[:], ps_ap)
            if sq_inplace:
                sq = ps_ap
            else:
                sq = ps.tile([128, IB], F32, tag="fill", name="sq")[:]
            nc.tensor.matmul(sq, lhsT=perm[:], rhs=raw[:],
                             start=True, stop=True, skip_group_check=True)
            tmp = rot.tile([128, IB], BF16, tag="tmp", name="tmp")
            nc.vector.tensor_mul(out=tmp[:], in0=raw[:], in1=crep[:, tcl])
            nc.vector.tensor_mul(out=dsl, in0=sq, in1=srep[:, tcl])
            nc.vector.tensor_add(out=dsl, in0=dsl, in1=tmp[:])

        def gen_kproj(hb, tc4):
            st = {}
            tcl = slice(tc4 * IB, (tc4 + 1) * IB)

            def mm(c0, c1):
                if c0 == 0:
                    st["ps"] = ps.tile([128, IB], F32, tag="fill", name="psk")
                for c in range(c0, c1):
                    nc.tensor.matmul(st["ps"][:],
                                     lhsT=wk_sb[:, c, hb * 128:(hb + 1) * 128],
                                     rhs=xnT[c][:, tcl],
                                     start=(c == 0), stop=(c == n_ct - 1))

            def fin():
                rotary(st["ps"][:], kt[hb][:, tcl], tcl)

            return [lambda: mm(0, 4), lambda: mm(4, 8), lambda: mm(8, 12),
                    lambda: mm(12, 16), fin]

        def gen_qproj(ib, hb):
            st = {}
            tcl = slice(ib * IB, (ib + 1) * IB)

            def dma():
                slab = wqp.tile([128, n_ct, 128], BF16, tag="wq", name="wqs")
                dmae[(ib + hb) % 2].dma_start(
                    slab[:],
                    wq_d[:, hb * 128:(hb + 1) * 128].rearrange("(c p) m -> p c m",
                                                               p=128))
                st["slab"] = slab

            def mm(c0, c1):
                if c0 == 0:
                    st["ps"] = ps.tile([128, IB], F32, tag="fill", name="psq")
                for c in range(c0, c1):
                    nc.tensor.matmul(st["ps"][:], lhsT=st["slab"][:, c, :],
                                     rhs=xnT[c][:, tcl],
                                     start=(c == 0), stop=(c == n_ct - 1))

            def fin():
                qt_t = qtp.tile([128, IB], BF16, tag=f"qt{hb}", name=f"qt{hb}")
                qt_slot[(ib, hb)] = qt_t
                rotary(st["ps"][:], qt_t[:], tcl)

            return [dma, lambda: mm(0, 4), lambda: mm(4, 8),
                    lambda: mm(8, 12), lambda: mm(12, 16), fin]

        def gen_vproj(t):
            st = {}

            def mm(c0, c1):
                if c0 == 0:
                    st["ps"] = ps.tile([128, IB], F32, tag="fill", name="psv")
                for c in range(c0, c1):
                    nc.tensor.matmul(st["ps"][:],
                                     lhsT=xnT[c][:, t * 128:(t + 1) * 128],
                                     rhs=wv_sb[:, c, :],
                                     start=(c == 0), stop=(c == n_ct - 1))

            def fin():
                nc.vector.tensor_copy(
                    vst[t][:, :, 0:DH],
                    st["ps"][:].rearrange("p (h d) -> p h d", h=HEADS))

            return [lambda: mm(0, 4), lambda: mm(4, 8), lambda: mm(8, 12),
                    lambda: mm(12, 16), fin]

        qt_slot = {}
        ont_ap = {}

        def gen_outproj(i, tail=False):
            """Wout projection of i-block i; bf16 [128, 2048] rows, 1 DMA per
            512-col chunk.  tail=True (final i-block, attention done): borrow
            the idle scores PSUM pool for 4-deep group pipelining and lag the
            hp=3 matmul (its normalize is still in flight) behind batches of
            hp<3 partial sums so the in-order PE queue never stalls on it."""
            ocs = [{} for _ in range(4)]
            shared = {}
            psos = {}

            def mk_pso(t):
                if tail:
                    if t % 2 == 0:
                        shared["scp"] = ps_sc.tile([128, 2, IB], F32,
                                                   tag="scp", name="tailps")
                    return shared["scp"][:, t % 2, :]
                return ps.tile([128, 512], F32, tag="fill", name="pso")[:]

            def mms(t, hps):
                ic, ec = t // 4, t % 4
                if hps[0] == 0:
                    psos[t] = mk_pso(t)
                for hp in hps:
                    nc.tensor.matmul(
                        psos[t],
                        lhsT=ont_ap[(i, hp)][:, ic * 128:(ic + 1) * 128],
                        rhs=wout_sb[:, hp, ec * 512:(ec + 1) * 512],
                        start=(hp == 0), stop=(hp == n_hb - 1),
                        skip_group_check=tail)

            def drain(t):
                ic, ec = t // 4, t % 4
                st = ocs[ic]
                if ec == 0:
                    st["oc"] = ocp.tile([128, DIM], BF16, tag="oc", name="oc")
                csl = slice(ec * 512, (ec + 1) * 512)
                nc.any.tensor_copy(st["oc"][:, csl], psos.pop(t))
                r0 = i * IB + ic * 128
                dmae[(i + ic + ec) % 2].dma_start(
                    out=out_d[r0:r0 + 128, csl], in_=st["oc"][:, csl])

            thunks = []
            if not tail:
                for t in range(16):
                    def th(t=t):
                        mms(t, range(n_hb))
                        drain(t)
                    thunks.append(th)
            else:
                for t in range(4):
                    thunks.append(lambda t=t: mms(t, range(n_hb - 1)))
                for tp in range(0, 16, 2):
                    def th(tp=tp):
                        for t in (tp, tp + 1):
                            mms(t, [n_hb - 1])
                            drain(t)
                        for t in (tp + 4, tp + 5):
                            if t < 16:
                                mms(t, range(n_hb - 1))
                    thunks.append(th)
            return thunks

        def pipeline_units(units):
            """Flatten unit thunk-lists, delaying each unit's fin until after
            the next unit's first matmul thunk (so the fin's PE perm-matmul
            never head-of-line blocks the tensor queue on the DVE drain)."""
            seq = []
            pending = None
            for u in units:
                *body, fin = u
                seq.append(body[0])
                if pending is not None:
                    seq.append(pending)
                seq += body[1:]
                pending = fin
            if pending is not None:
                seq.append(pending)
            return seq

        # ---- phase 1 wave 0: 5 K-projection units (4 head-pairs x tokens
        # 0:512 plus head-pair 0 x tokens 512:1024) round-robin per c-chunk
        # across 5 PSUM accumulators (4 borrowed from the idle scores pool)
        # so the PE tracks the incoming xnT DMA stream instead of
        # head-of-line blocking on each unit's full c-sweep.
        kacc_scp = [ps_sc.tile([128, 2, IB], F32, tag="scp", name=f"kacc{j}")
                    for j in (0, 1)]
        kacc_f = ps.tile([128, IB], F32, tag="fill", name="kacc4")
        wave = [(kacc_scp[0][:, 0, :], 0, 0), (kacc_scp[0][:, 1, :], 1, 0),
                (kacc_scp[1][:, 0, :], 2, 0), (kacc_scp[1][:, 1, :], 3, 0),
                (kacc_f[:], 0, 1)]
        for c in range(n_ct):
            for acc, hb, tc4 in wave:
                nc.tensor.matmul(acc,
                                 lhsT=wk_sb[:, c, hb * 128:(hb + 1) * 128],
                                 rhs=xnT[c][:, tc4 * IB:(tc4 + 1) * IB],
                                 start=(c == 0), stop=(c == n_ct - 1))
        # ---- phase 1: K (rest), V (first 4), Q (i-block 0), with the
        # wave's rotary fins (DVE-heavy, allocation-free via sq_inplace)
        # interleaved so the PE chews projection matmuls while the DVE
        # drains the wave.  The fill-ring accumulator's fin (wave[4]) must
        # precede the second rest unit's PSUM allocation.
        units = [gen_kproj(hb, tc4) for hb in range(n_hb)
                 for tc4 in range(1, n_ib) if (hb, tc4) != (0, 1)]
        units += [gen_vproj(t) for t in range(jpi)]
        units += [gen_qproj(0, hb) for hb in range(n_hb)]
        rest = pipeline_units(units)
        wave_fins = {0: 4, 3: 0, 6: 1, 9: 2, 12: 3}  # rest-index -> wave idx
        for j, th in enumerate(rest):
            th()
            if j in wave_fins:
                acc, hb, tc4 = wave[wave_fins[j]]
                tcl = slice(tc4 * IB, (tc4 + 1) * IB)
                rotary(acc, kt[hb][:, tcl], tcl, sq_inplace=True)

        # ---- phase 2: attention with pipelined filler ----
        pending = [None]
        for i in range(n_ib):
            funits = []
            if i < n_ib - 1:
                for hb in range(n_hb):
                    funits.append(gen_qproj(i + 1, hb))
                for t in range(jpi * (i + 1), jpi * (i + 2)):
                    funits.append(gen_vproj(t))
            fillers = pipeline_units(funits) if funits else []
            if i >= 2:
                fillers += gen_outproj(i - 2)
            if i == n_ib - 1:
                fillers += gen_outproj(i - 1)

            n_jb = jpi * (i + 1)
            total_steps = n_hb * n_jb
            done = 0
            step = 0
            for hp in range(n_hb):
                o_ps = [ps_av.tile([DH + 1, IB], F32, tag=f"oav{k}",
                                   name=f"oav{k}") for k in (0, 1)]
                for jb in range(n_jb):
                    delta = jb - jpi * i
                    v0 = max(delta, 0) * 128
                    psl = slice(v0, IB)
                    jsl = slice(jb * 128, (jb + 1) * 128)
                    diag = delta >= 0
                    s_pair = ps_sc.tile([128, 2, IB], F32, tag="scp",
                                        name="scp")
                    # both heads' scores back-to-back: disjoint PE row
                    # groups (C=64 at rows 0:64 / 64:128) -> run concurrent
                    for k in (0, 1):
                        hsl = slice(k * 64, (k + 1) * 64)
                        nc.tensor.matmul(s_pair[:, k, psl],
                                         lhsT=kt[hp][hsl, jsl],
                                         rhs=qt_slot[(i, hp)][hsl, psl],
                                         start=True, stop=not diag)
                    if diag:
                        # causal mask: accumulate -3e29 into the strict
                        # lower triangle of the diagonal 128x128 block
                        dsl = slice(v0, v0 + 128)
                        for k in (0, 1):
                            nc.tensor.matmul(s_pair[:, k, dsl], lhsT=eye[:],
                                             rhs=trineg[:],
                                             start=False, stop=True,
                                             skip_group_check=True)
                    e_t = epool.tile([128, 2, IB], BF16, tag="e", name="e")
                    if diag:
                        # both heads' clipped regions in one strided ACTIVATE
                        nc.scalar.activation(out=e_t[:, :, psl],
                                             in_=s_pair[:, :, psl],
                                             func=AF.Exp, scale=scale)
                    else:
                        nc.scalar.activation(out=e_t[:, :, :],
                                             in_=s_pair[:, :, :],
                                             func=AF.Exp, scale=scale)
                    # cross-i-block flushes get an extra j-block of slack:
                    # the previous head-pair's reciprocal chain is longer
                    # than 2 j-blocks at an i-block transition.
                    if jb == (3 if hp == 0 else 2) and pending[0] is not None:
                        pending[0]()
                        pending[0] = None
                    # filler between scores and AV: absorbs the exp latency
                    # (front-load the first j-blocks of each head-pair where
                    # the pipeline is shallow)
                    step += 1
                    want = len(fillers) * step // total_steps
                    if jb < 2 and hp + i > 0:
                        want += 2
                    while done < min(want, len(fillers)):
                        fillers[done]()
                        done += 1
                    for k in (0, 1):
                        h = hp * 2 + k
                        nc.tensor.matmul(o_ps[k][:, psl],
                                         lhsT=vst[jb][:, h, :],
                                         rhs=e_t[:, k, psl],
                                         start=(jb == 0), stop=(jb == n_jb - 1))

                # AV psum -> one sbuf bf16 tile (k0 rows 0:64, k1 rows
                # 64:128 via the psum-source partition shift).  The [1,512]
                # denominator rows are spread over partitions {0,32,64,96}
                # so the DVE reciprocal runs 4 lanes wide.
                osb = osbp.tile([128, IB], BF16, tag="osb", name="osb")
                nc.vector.tensor_copy(osb[0:DH, :], o_ps[0][0:DH, :])
                nc.vector.tensor_copy(osb[64:128, :], o_ps[1][0:DH, :])
                # denominators: psum row 64 -> sbuf rows 0/32 (aligned), then
                # an SBUF->SBUF DMA spreads each [1,512] row over 4 adjacent
                # partitions so the reciprocal runs 8 DVE lanes wide.
                stage = denp.tile([128, IB], BF16, tag="dstage", name="dstage")
                nc.vector.tensor_copy(stage[0:1, :], o_ps[0][DH:DH + 1, :])
                nc.vector.tensor_copy(stage[32:33, :], o_ps[1][DH:DH + 1, :])
                denin = denp.tile([128, 128], BF16, tag="denin", name="denin")
                denf = denp.tile([128, 128], F32, tag="denf", name="denf")
                recb = denp.tile([128, 128], BF16, tag="recb", name="recb")
                dmae[hp % 2].dma_start(out=denin[0:4, :], in_=stage[0:1, :])
                dmae[(hp + 1) % 2].dma_start(out=denin[4:8, :],
                                             in_=stage[32:33, :])
                nc.vector.reciprocal(out=denf[0:8, :], in_=denin[0:8, :])
                nc.vector.tensor_copy(recb[0:8, :], denf[0:8, :])

                def norm(i=i, hp=hp, osb=osb, recb=recb):
                    # deferred into the next head-pair's j-loop so the pbc
                    # matmuls never head-of-line block on the DVE chain
                    ont_t = ontp.tile([128, IB], BF16, tag=f"ont{hp}",
                                      name=f"ont{hp}")
                    ont_ap[(i, hp)] = ont_t
                    pbc = ps.tile([128, IB], F32, tag="fill", name="pbc")
                    for o in range(4):
                        for k in (0, 1):   # k-adjacent: col-group concurrent
                            nc.tensor.matmul(
                                pbc[64 * k:64 * k + 64,
                                    o * 128:(o + 1) * 128],
                                lhsT=sel8[0:8,
                                          64 * (4 * k + o):
                                          64 * (4 * k + o) + 64],
                                rhs=recb[0:8, :],
                                start=True, stop=True,
                                tile_position=(0, 64 * k),
                                skip_group_check=True)
                    nc.vector.tensor_mul(out=ont_t[:], in0=pbc[:],
                                         in1=osb[:])
                pending[0] = norm

            while done < len(fillers):
                fillers[done]()
                done += 1

        # tail: the hp<3 partial sums have no dependency on the last
        # head-pair's normalize -- run them first so the PE stays busy (and
        # the HAM clock stays warm) while the norm's DVE/DMA chain drains.
        tail_thunks = gen_outproj(n_ib - 1, tail=True)
        for th in tail_thunks[:4]:
            th()
        # HAM warm-keepers: dependency-free junk matmuls bridge the last
        # normalize's DVE/DMA chain so the final out-projection doesn't
        # drop to the cold 1.2 GHz clock.
        warm = ps.tile([128, IB], F32, tag="fill", name="warm")
        for _ in range(22):
            nc.tensor.matmul(warm[:, 0:128], lhsT=eye[:], rhs=trineg[:],
                             start=True, stop=True, skip_group_check=True)
        if pending[0] is not None:
            pending[0]()
            pending[0] = None
        for th in tail_thunks[4:]:
            th()

    nc.compile()
    return nc


def get_nc():
    if "nc" not in _CACHED:
        _CACHED["nc"] = _build()
    return _CACHED["nc"]


def host_inputs(x, rotary_emb, gamma, Wq, Wkv, Wout):
    """Build the 8 per-core input dicts."""
    bf = ml_dtypes.bfloat16
    x = np.asarray(x, np.float32)
    g = np.asarray(gamma, np.float32)
    nrm = np.sqrt((x * x).sum(-1, keepdims=True))
    xn = x / np.maximum(nrm, 1e-12) * (DIM ** 0.5) * g
    Wq = np.asarray(Wq, np.float32)
    Wkv = np.asarray(Wkv, np.float32)
    Wk = Wkv[:, :HEADS_TOTAL * DH]
    Wv = Wkv[:, HEADS_TOTAL * DH:]
    Wout = np.asarray(Wout, np.float32)
    pos = np.asarray(rotary_emb, np.float32)
    cos = np.cos(pos).T
    sgn = np.concatenate([-np.ones(DH // 2), np.ones(DH // 2)]).astype(np.float32)
    sin = (np.sin(pos) * sgn[None, :]).T
    cosr = np.ascontiguousarray(np.tile(cos, (2, 1)).astype(bf))
    sinr = np.ascontiguousarray(np.tile(sin, (2, 1)).astype(bf))
    jj, ii = np.mgrid[0:128, 0:128]
    trineg = np.ascontiguousarray(
        np.where(ii < jj, np.float32(-3e29), np.float32(0)).astype(bf))
    eye = np.ascontiguousarray(np.eye(128, dtype=np.float32).astype(bf))
    # row-selector for the 1/d broadcast matmuls: column block s reads
    # denominator row s (chunk o of head k lives at partition 4k+o)
    pp, cc = np.mgrid[0:128, 0:512]
    sel8 = np.ascontiguousarray((pp == cc // 64).astype(np.float32).astype(bf))
    # rotate_half partition permutation: out row p reads row sigma(p); sigma
    # swaps the 32-halves within each 64-row head slot.
    sigma = (np.arange(128) % 64 // 32 * -64 + 32) + np.arange(128)
    perm = np.zeros((128, 128), np.float32)
    perm[sigma, np.arange(128)] = 1.0
    perm = np.ascontiguousarray(perm.astype(bf))
    maps = []
    for core in range(N_CORES):
        b, gq = core // GROUPS, core % GROUPS
        hs = slice(gq * HD, (gq + 1) * HD)
        maps.append({
            "xnT": np.ascontiguousarray(xn[b].T.astype(bf)),
            "wq": np.ascontiguousarray(Wq[:, hs].astype(bf)),
            "wk": np.ascontiguousarray(Wk[:, hs].astype(bf)),
            "wv": np.ascontiguousarray(Wv[:, hs].astype(bf)),
            "wout": np.ascontiguousarray(Wout[hs, :].astype(bf)),
            "cosr": cosr, "sinr": sinr, "trineg": trineg, "eye": eye,
            "sel8": sel8, "perm": perm,
        })
    return maps


def run_cores(in_maps, trace=False, **kwargs):
    from concourse.bass_utils import run_bass_kernel_spmd
    nc = get_nc()
    return run_bass_kernel_spmd(nc, in_maps, list(range(N_CORES)), trace=trace,
                                **kwargs)


def kernel(x, rotary_emb, gamma, Wq, Wkv, Wout):
    in_maps = host_inputs(x, rotary_emb, gamma, Wq, Wkv, Wout)
    res = run_cores(in_maps, trace=False)
    out = np.zeros((B, N, DIM), np.float32)
    for core in range(N_CORES):
        b = core // GROUPS
        out[b] += res.results[core]["out"].astype(np.float32)
    return out
